# revision 12
# baseline (speedup 1.0000x reference)
"""Trainium2 Bass kernel for STSBaselineNet (embed -> biLSTM -> max-pool).

Sharding v3: one LSTM direction per core (cores 0-3 fwd, 4-7 bwd, 64
sentences each). The input projection emb @ W_ih^T + b is folded on the host
into a [V+1, 1024] bf16 gather table per direction (row V = pad row that
drives the gates to freeze-ish values), so the device does:

  Phase A: indirect-DMA gather of 4096 zx rows (token-major), then
           DMA-engine transposes (InstDmaTransposeAnt) into gate-major
           zxT[128, s*512 + ch*64 + b] - no PE/DVE involvement.
  Phase B: 64-step recurrence, gates on partitions, batch on free dim.
           Two 32-sentence half-chains ping-pong so the elementwise chain of
           one half hides under the matmul block of the other. Gate chunk
           order [i0 i1 f0 f1 o0 o1 g0 g1]; tanh is computed via the
           sigmoid-only identity tanh(x) = 2*sigmoid(2x)-1 (the 2x is folded
           into the g rows of the table and W_hh on the host), so each half
           needs just two ACTIVATE ops. Running max instead of storing h.
  Phase C: transpose the running max, DMA out [64, 256].

Scan order: token s of chain b reads token_ids[b, s] (fwd) or
token_ids[b, len_b-1-s] (bwd), pad row for s >= len_b, so pads always trail
and both directions share the same program.
"""

import numpy as np
import ml_dtypes

import concourse.bass as bass
import concourse.bacc as bacc
import concourse.mybir as mybir
import concourse.tile as tile
from concourse import bass_utils

V, E, HID, B, T = 50000, 300, 256, 256, 64
NCORES = 8
NSC = 64                    # sentences per core (one direction)
NTOK = NSC * T              # 4096 tokens/core
NTT = NTOK // 128           # 32 gather tiles
G4 = 4 * HID                # 1024 gate logits
VP = V + 1                  # table rows (last = pad row)
BIG = 30.0

F32 = mybir.dt.float32
BF16 = mybir.dt.bfloat16
I32 = mybir.dt.int32
AF = mybir.ActivationFunctionType
OP = mybir.AluOpType

bf = ml_dtypes.bfloat16

_CACHE = {}
LAST_RESULTS = None


def _build_program():
    nc = bacc.Bacc(None, target_bir_lowering=False)

    tab_d = nc.dram_tensor("tab", [VP, G4], BF16, kind="ExternalInput")
    idx_d = nc.dram_tensor("idx", [128, NTT], I32, kind="ExternalInput")
    wstat_d = nc.dram_tensor("wstat", [128, 2048], BF16, kind="ExternalInput")
    out_d = nc.dram_tensor("out", [NSC, HID], F32, kind="ExternalOutput")

    with tile.TileContext(nc) as tc:
        with (
            tc.tile_pool(name="const", bufs=1) as cpool,
            tc.tile_pool(name="stage", bufs=4) as spool,
            tc.tile_pool(name="work", bufs=3) as wpool,
            tc.tile_pool(name="psum", bufs=2, space="PSUM") as ppool,
            tc.tile_pool(name="psumt", bufs=1, space="PSUM") as tpool,
        ):
            wstat_sb = cpool.tile([128, 2048], BF16, tag="wstat")
            idx_sb = cpool.tile([128, NTT], I32, tag="idx")
            zxT = cpool.tile([128, T * 512], BF16, tag="zxT")
            h_pp = [cpool.tile([128, 128], BF16, tag=f"h{i}", name=f"h{i}")
                    for i in range(2)]
            c_st = cpool.tile([128, 128], F32, tag="c_st")
            rmax = cpool.tile([128, 128], BF16, tag="rmax")
            ident = cpool.tile([128, 128], F32, tag="ident")
            ident_bf = cpool.tile([128, 128], BF16, tag="ident_bf")
            outT = cpool.tile([128, 128], F32, tag="outT")

            nc.sync.dma_start(out=wstat_sb[:], in_=wstat_d[:, :])
            nc.sync.dma_start(out=idx_sb[:], in_=idx_d[:, :])

            nc.vector.memset(c_st[:], 0.0)
            nc.vector.memset(h_pp[0][:], 0.0)
            nc.vector.memset(h_pp[1][:], 0.0)
            nc.vector.memset(rmax[:], -1.0)
            from concourse.masks import make_identity
            make_identity(nc, ident[:])
            nc.vector.tensor_copy(out=ident_bf[:], in_=ident[:])

            # ---- Phase A: gather + DMA-transpose to gate-major ----
            # zxT col = s*512 + ch*64 + (k*32 + b_h)  (same (k,b) layout as
            # the psum gate chunks). Gather tile u holds tokens
            # j = u*128 + p with j = s*64 + b  ->  s = 2u + p//64, b = p%64.
            _z = zxT[:]
            for u in range(NTT):
                st = spool.tile([128, G4], BF16, tag="st")
                nc.gpsimd.indirect_dma_start(
                    out=st[:],
                    out_offset=None,
                    in_=tab_d[:, :],
                    in_offset=bass.IndirectOffsetOnAxis(
                        ap=idx_sb[:, u:u + 1], axis=0),
                )
                for ch in range(8):
                    nc.sync.dma_start_transpose(
                        out=zxT[:, u * 1024 + ch * 128:
                                u * 1024 + (ch + 1) * 128],
                        in_=st[:, ch * 128:(ch + 1) * 128])

            # ---- Phase B: recurrence ----
            for s in range(T):
                h_prev = h_pp[(s + 1) % 2]
                h_cur = h_pp[s % 2]
                for half in range(2):
                    co = half * 64
                    zqb = ppool.tile([128, 256], F32, tag=f"zqb{half}")
                    for ch in range(8):
                        for k in range(2):
                            rhs = h_prev[:, co + k * 32:co + k * 32 + 32]
                            nc.tensor.matmul(
                                zqb[:, ch * 32:(ch + 1) * 32],
                                lhsT=wstat_sb[:, (ch * 2 + k) * 128:
                                              (ch * 2 + k + 1) * 128],
                                rhs=rhs,
                                start=(k == 0), stop=(k == 1),
                            )
                    # elementwise: zs = zqb + zx; all-sigmoid gates
                    zs = wpool.tile([128, 256], F32, tag=f"zs{half}")
                    # zxT col = (s//2)*1024 + ch*128 + (s%2)*64 + b
                    zx_ap = bass.AP(
                        tensor=_z.tensor,
                        offset=(_z.offset + (s // 2) * 1024 + (s % 2) * 64
                                + half * 32),
                        ap=[_z.ap[0], [128, 8], [1, 32]])
                    zq_v = zqb[:].rearrange("p (c g) -> p c g", c=8)
                    nc.vector.tensor_add(
                        zs[:].rearrange("p (c g) -> p c g", c=8),
                        zq_v, zx_ap)
                    sg = wpool.tile([128, 256], F32, tag=f"sg{half}")
                    nc.scalar.activation(sg[:, 0:192], zs[:, 0:192], AF.Sigmoid)
                    nc.scalar.activation(sg[:, 192:256], zs[:, 192:256],
                                         AF.Tanh)
                    tig = wpool.tile([128, 64], F32, tag=f"tig{half}")
                    nc.vector.tensor_mul(tig[:], sg[:, 0:64], sg[:, 192:256])
                    cc = c_st[:, co:co + 64]
                    nc.vector.tensor_mul(cc, cc, sg[:, 64:128])
                    nc.vector.tensor_add(cc, cc, tig[:])
                    th = wpool.tile([128, 64], F32, tag=f"th{half}")
                    nc.scalar.activation(th[:], cc, AF.Tanh)
                    hh = h_cur[:, co:co + 64]
                    nc.vector.tensor_mul(hh, sg[:, 128:192], th[:])
                    nc.vector.tensor_max(rmax[:, co:co + 64],
                                         rmax[:, co:co + 64], hh)

            # ---- Phase C: transpose running max, write out ----
            tp = tpool.tile([128, 128], BF16, tag="tp")
            nc.tensor.transpose(tp[:], rmax[:], ident_bf[:])
            nc.vector.tensor_copy(out=outT[:], in_=tp[:])
            # outT[j = half*64 + k*32 + b, p] -> out[half*32 + b, k*128 + p]
            for half in range(2):
                for k in range(2):
                    out_ap = bass.AP(
                        tensor=out_d[:, :].tensor,
                        offset=(half * 32) * HID + k * 128,
                        ap=[[HID, 32], [1, 128]])
                    nc.sync.dma_start(
                        out=out_ap,
                        in_=outT[half * 64 + k * 32:half * 64 + k * 32 + 32, :])

    nc.finalize()
    return nc


def _host_prep(token_ids, lengths, emb, w_ih_f, w_hh_f, b_f, w_ih_b, w_hh_b,
               b_b):
    # gate chunk order [i0 i1 f0 f1 o0 o1 g0 g1] as torch rows
    ch_rows = [0, 128, 256, 384, 768, 896, 512, 640]
    col_perm = np.concatenate([np.arange(r, r + 128) for r in ch_rows])

    tabs, wstats = {}, {}
    for d in range(2):
        w_ih = w_ih_f if d == 0 else w_ih_b
        whh = w_hh_f if d == 0 else w_hh_b
        bias = b_f if d == 0 else b_b

        zx = emb.astype(np.float32) @ w_ih.T.astype(np.float32) + bias
        tab = np.empty((VP, G4), dtype=bf)
        tab[:V] = zx[:, col_perm].astype(bf)
        padv = np.empty(G4, dtype=np.float32)       # in permuted chunk order
        padv[0:256] = BIG                           # i -> 1
        padv[256:512] = -BIG                        # f -> 0
        padv[512:768] = BIG                         # o -> 1
        padv[768:1024] = -BIG                       # g -> tanh = -1
        tab[V] = padv.astype(bf)
        tabs[d] = tab

        whh2 = whh.astype(np.float32)
        wstat = np.zeros((128, 2048), dtype=bf)
        for ch in range(8):
            for k in range(2):
                blk = whh2[ch_rows[ch]:ch_rows[ch] + 128,
                           k * 128:(k + 1) * 128].T
                col = (ch * 2 + k) * 128
                wstat[:, col:col + 128] = blk.astype(bf)
        wstats[d] = wstat

    in_maps = []
    for c in range(NCORES):
        d = 0 if c < 4 else 1
        blk = c % 4
        tok = token_ids[blk * NSC:(blk + 1) * NSC]      # [64, 64]
        ln = lengths[blk * NSC:(blk + 1) * NSC]         # [64]

        ss = np.arange(T)[None, :]                      # [1, T]
        if d == 0:
            pos = ss                                    # fwd: s
        else:
            pos = ln[:, None] - 1 - ss                  # bwd: len-1-s
        valid = ss < ln[:, None]                        # [64, T]
        rows = np.where(valid, np.take_along_axis(
            tok, np.clip(pos, 0, T - 1), axis=1), V)    # [64, T] table rows
        flat = rows.T.reshape(-1)                       # j = s*64 + b
        idx = flat.reshape(NTT, 128).T.astype(np.int32).copy()

        in_maps.append({
            "tab": tabs[d],
            "idx": idx,
            "wstat": wstats[d],
        })
    return in_maps


def kernel(token_ids, lengths, emb, w_ih_f, w_hh_f, b_f, w_ih_b, w_hh_b, b_b):
    global LAST_RESULTS
    if "nc" not in _CACHE:
        _CACHE["nc"] = _build_program()
    nc = _CACHE["nc"]
    in_maps = _host_prep(token_ids, lengths, emb, w_ih_f, w_hh_f, b_f,
                         w_ih_b, w_hh_b, b_b)
    res = bass_utils.run_bass_kernel_spmd(nc, in_maps, list(range(NCORES)))
    LAST_RESULTS = res
    out = np.zeros((B, 2 * HID), np.float32)
    for c in range(NCORES):
        d = 0 if c < 4 else 1
        blk = c % 4
        out[blk * NSC:(blk + 1) * NSC,
            d * HID:(d + 1) * HID] = res.results[c]["out"]
    return out


# revision 15
# speedup vs baseline: 1.9993x; 1.9993x over previous
"""Trainium2 Bass kernel for STSBaselineNet (embed -> biLSTM -> max-pool).

Sharding v3: one LSTM direction per core (cores 0-3 fwd, 4-7 bwd, 64
sentences each). The input projection emb @ W_ih^T + b is folded on the host
into a [V+1, 1024] bf16 gather table per direction (row V = pad row that
drives the gates to freeze-ish values), so the device does:

  Phase A: indirect-DMA gather of 4096 zx rows (token-major), then
           DMA-engine transposes (InstDmaTransposeAnt) into gate-major
           zxT[128, s*512 + ch*64 + b] - no PE/DVE involvement.
  Phase B: 64-step recurrence, gates on partitions, batch on free dim.
           Two 32-sentence half-chains ping-pong so the elementwise chain of
           one half hides under the matmul block of the other. Gate chunk
           order [i0 i1 f0 f1 o0 o1 g0 g1]; tanh is computed via the
           sigmoid-only identity tanh(x) = 2*sigmoid(2x)-1 (the 2x is folded
           into the g rows of the table and W_hh on the host), so each half
           needs just two ACTIVATE ops. Running max instead of storing h.
  Phase C: transpose the running max, DMA out [64, 256].

Scan order: token s of chain b reads token_ids[b, s] (fwd) or
token_ids[b, len_b-1-s] (bwd), pad row for s >= len_b, so pads always trail
and both directions share the same program.
"""

import numpy as np
import ml_dtypes

import concourse.bass as bass
import concourse.bacc as bacc
import concourse.mybir as mybir
import concourse.tile as tile
from concourse import bass_utils

V, E, HID, B, T = 50000, 300, 256, 256, 64
NCORES = 8
NSC = 64                    # sentences per core (one direction)
NTOK = NSC * T              # 4096 tokens/core
NTT = NTOK // 128           # 32 gather tiles
G4 = 4 * HID                # 1024 gate logits
VP = V + 1                  # table rows (last = pad row)
BIG = 30.0

F32 = mybir.dt.float32
BF16 = mybir.dt.bfloat16
I32 = mybir.dt.int32
AF = mybir.ActivationFunctionType
OP = mybir.AluOpType

bf = ml_dtypes.bfloat16

_CACHE = {}
LAST_RESULTS = None


def _build_program():
    nc = bacc.Bacc(None, target_bir_lowering=False)

    tab_d = nc.dram_tensor("tab", [VP, G4], BF16, kind="ExternalInput")
    idx_d = nc.dram_tensor("idx", [128, NTT], I32, kind="ExternalInput")
    wstat_d = nc.dram_tensor("wstat", [128, 2048], BF16, kind="ExternalInput")
    out_d = nc.dram_tensor("out", [NSC, HID], F32, kind="ExternalOutput")

    with tile.TileContext(nc) as tc:
        with (
            tc.tile_pool(name="const", bufs=1) as cpool,
            tc.tile_pool(name="stage", bufs=4) as spool,
            tc.tile_pool(name="work", bufs=3) as wpool,
            tc.tile_pool(name="psum", bufs=2, space="PSUM") as ppool,
            tc.tile_pool(name="psumt", bufs=2, space="PSUM") as tpool,
        ):
            wstat_sb = cpool.tile([128, 2048], BF16, tag="wstat")
            idx_sb = cpool.tile([128, NTT], I32, tag="idx")
            zxT = cpool.tile([128, T * 512], BF16, tag="zxT")
            h_pp = [cpool.tile([128, 128], BF16, tag=f"h{i}", name=f"h{i}")
                    for i in range(2)]
            c_st = cpool.tile([128, 128], F32, tag="c_st")
            rmax = cpool.tile([128, 128], BF16, tag="rmax")
            ident = cpool.tile([128, 128], F32, tag="ident")
            ident_bf = cpool.tile([128, 128], BF16, tag="ident_bf")
            outT = cpool.tile([128, 128], F32, tag="outT")

            nc.sync.dma_start(out=wstat_sb[:], in_=wstat_d[:, :])
            nc.sync.dma_start(out=idx_sb[:], in_=idx_d[:, :])

            nc.vector.memset(c_st[:], 0.0)
            nc.vector.memset(h_pp[0][:], 0.0)
            nc.vector.memset(h_pp[1][:], 0.0)
            nc.vector.memset(rmax[:], -1.0)
            from concourse.masks import make_identity
            make_identity(nc, ident[:])
            nc.vector.tensor_copy(out=ident_bf[:], in_=ident[:])

            # ---- Phase A: gather + PE-transpose to gate-major ----
            # zxT col = (s//2)*1024 + ch*128 + (s%2)*64 + b. Gather tile u
            # holds tokens j = u*128 + p, j = s*64 + b. The 8 PE transposes
            # per tile batch through one PSUM bank, then one DMA moves the
            # bank to zxT. Emission is interleaved into the step loop so the
            # in-order PE queue treats transposes as inter-block filler.
            _z = zxT[:]

            def emit_tile(u):
                st = spool.tile([128, G4], BF16, tag="st", name="st")
                nc.gpsimd.indirect_dma_start(
                    out=st[:],
                    out_offset=None,
                    in_=tab_d[:, :],
                    in_offset=bass.IndirectOffsetOnAxis(
                        ap=idx_sb[:, u:u + 1], axis=0),
                )
                tb = tpool.tile([128, 1024], BF16, tag="tb", name="tb")
                for ch in range(8):
                    nc.tensor.transpose(
                        tb[:, ch * 128:(ch + 1) * 128],
                        st[:, ch * 128:(ch + 1) * 128],
                        ident_bf[:])
                nc.scalar.copy(
                    out=zxT[:, u * 1024:(u + 1) * 1024], in_=tb[:])

            LOOKAHEAD = 4
            for u in range(LOOKAHEAD):
                emit_tile(u)

            # ---- Phase B: recurrence ----
            for s in range(T):
                if s % 2 == 0 and s // 2 + LOOKAHEAD < NTT:
                    emit_tile(s // 2 + LOOKAHEAD)
                h_prev = h_pp[(s + 1) % 2]
                h_cur = h_pp[s % 2]
                for half in range(2):
                    co = half * 64
                    zqb = ppool.tile([128, 256], F32, tag=f"zqb{half}")
                    for ch in range(8):
                        for k in range(2):
                            rhs = h_prev[:, co + k * 32:co + k * 32 + 32]
                            nc.tensor.matmul(
                                zqb[:, ch * 32:(ch + 1) * 32],
                                lhsT=wstat_sb[:, (ch * 2 + k) * 128:
                                              (ch * 2 + k + 1) * 128],
                                rhs=rhs,
                                start=(k == 0), stop=(k == 1),
                            )
                    # elementwise: zs = zqb + zx; all-sigmoid gates
                    zs = wpool.tile([128, 256], F32, tag=f"zs{half}")
                    # zxT col = (s//2)*1024 + ch*128 + (s%2)*64 + b
                    zx_ap = bass.AP(
                        tensor=_z.tensor,
                        offset=(_z.offset + (s // 2) * 1024 + (s % 2) * 64
                                + half * 32),
                        ap=[_z.ap[0], [128, 8], [1, 32]])
                    zq_v = zqb[:].rearrange("p (c g) -> p c g", c=8)
                    nc.vector.tensor_add(
                        zs[:].rearrange("p (c g) -> p c g", c=8),
                        zq_v, zx_ap)
                    sg = wpool.tile([128, 256], F32, tag=f"sg{half}")
                    nc.scalar.activation(sg[:, 0:192], zs[:, 0:192], AF.Sigmoid)
                    nc.scalar.activation(sg[:, 192:256], zs[:, 192:256],
                                         AF.Tanh)
                    tig = wpool.tile([128, 64], F32, tag=f"tig{half}")
                    nc.vector.tensor_mul(tig[:], sg[:, 0:64], sg[:, 192:256])
                    cc = c_st[:, co:co + 64]
                    nc.vector.tensor_mul(cc, cc, sg[:, 64:128])
                    nc.vector.tensor_add(cc, cc, tig[:])
                    th = wpool.tile([128, 64], F32, tag=f"th{half}")
                    nc.scalar.activation(th[:], cc, AF.Tanh)
                    hh = h_cur[:, co:co + 64]
                    nc.vector.tensor_mul(hh, sg[:, 128:192], th[:])
                    nc.vector.tensor_max(rmax[:, co:co + 64],
                                         rmax[:, co:co + 64], hh)

            # ---- Phase C: transpose running max, write out ----
            tp = tpool.tile([128, 128], BF16, tag="tp")
            nc.tensor.transpose(tp[:], rmax[:], ident_bf[:])
            nc.vector.tensor_copy(out=outT[:], in_=tp[:])
            # outT[j = half*64 + k*32 + b, p] -> out[half*32 + b, k*128 + p]
            for half in range(2):
                for k in range(2):
                    out_ap = bass.AP(
                        tensor=out_d[:, :].tensor,
                        offset=(half * 32) * HID + k * 128,
                        ap=[[HID, 32], [1, 128]])
                    nc.sync.dma_start(
                        out=out_ap,
                        in_=outT[half * 64 + k * 32:half * 64 + k * 32 + 32, :])

    nc.finalize()
    return nc


def _host_prep(token_ids, lengths, emb, w_ih_f, w_hh_f, b_f, w_ih_b, w_hh_b,
               b_b):
    # gate chunk order [i0 i1 f0 f1 o0 o1 g0 g1] as torch rows
    ch_rows = [0, 128, 256, 384, 768, 896, 512, 640]
    col_perm = np.concatenate([np.arange(r, r + 128) for r in ch_rows])

    tabs, wstats = {}, {}
    for d in range(2):
        w_ih = w_ih_f if d == 0 else w_ih_b
        whh = w_hh_f if d == 0 else w_hh_b
        bias = b_f if d == 0 else b_b

        zx = emb.astype(np.float32) @ w_ih.T.astype(np.float32) + bias
        tab = np.empty((VP, G4), dtype=bf)
        tab[:V] = zx[:, col_perm].astype(bf)
        padv = np.empty(G4, dtype=np.float32)       # in permuted chunk order
        padv[0:256] = BIG                           # i -> 1
        padv[256:512] = -BIG                        # f -> 0
        padv[512:768] = BIG                         # o -> 1
        padv[768:1024] = -BIG                       # g -> tanh = -1
        tab[V] = padv.astype(bf)
        tabs[d] = tab

        whh2 = whh.astype(np.float32)
        wstat = np.zeros((128, 2048), dtype=bf)
        for ch in range(8):
            for k in range(2):
                blk = whh2[ch_rows[ch]:ch_rows[ch] + 128,
                           k * 128:(k + 1) * 128].T
                col = (ch * 2 + k) * 128
                wstat[:, col:col + 128] = blk.astype(bf)
        wstats[d] = wstat

    in_maps = []
    for c in range(NCORES):
        d = 0 if c < 4 else 1
        blk = c % 4
        tok = token_ids[blk * NSC:(blk + 1) * NSC]      # [64, 64]
        ln = lengths[blk * NSC:(blk + 1) * NSC]         # [64]

        ss = np.arange(T)[None, :]                      # [1, T]
        if d == 0:
            pos = ss                                    # fwd: s
        else:
            pos = ln[:, None] - 1 - ss                  # bwd: len-1-s
        valid = ss < ln[:, None]                        # [64, T]
        rows = np.where(valid, np.take_along_axis(
            tok, np.clip(pos, 0, T - 1), axis=1), V)    # [64, T] table rows
        flat = rows.T.reshape(-1)                       # j = s*64 + b
        idx = flat.reshape(NTT, 128).T.astype(np.int32).copy()

        in_maps.append({
            "tab": tabs[d],
            "idx": idx,
            "wstat": wstats[d],
        })
    return in_maps


def kernel(token_ids, lengths, emb, w_ih_f, w_hh_f, b_f, w_ih_b, w_hh_b, b_b):
    global LAST_RESULTS
    if "nc" not in _CACHE:
        _CACHE["nc"] = _build_program()
    nc = _CACHE["nc"]
    in_maps = _host_prep(token_ids, lengths, emb, w_ih_f, w_hh_f, b_f,
                         w_ih_b, w_hh_b, b_b)
    res = bass_utils.run_bass_kernel_spmd(nc, in_maps, list(range(NCORES)))
    LAST_RESULTS = res
    out = np.zeros((B, 2 * HID), np.float32)
    for c in range(NCORES):
        d = 0 if c < 4 else 1
        blk = c % 4
        out[blk * NSC:(blk + 1) * NSC,
            d * HID:(d + 1) * HID] = res.results[c]["out"]
    return out


# revision 18
# speedup vs baseline: 2.0102x; 1.0055x over previous
"""Trainium2 Bass kernel for STSBaselineNet (embed -> biLSTM -> max-pool).

Sharding v3: one LSTM direction per core (cores 0-3 fwd, 4-7 bwd, 64
sentences each). The input projection emb @ W_ih^T + b is folded on the host
into a [V+1, 1024] bf16 gather table per direction (row V = pad row that
drives the gates to freeze-ish values), so the device does:

  Phase A: indirect-DMA gather of 4096 zx rows (token-major), then
           DMA-engine transposes (InstDmaTransposeAnt) into gate-major
           zxT[128, s*512 + ch*64 + b] - no PE/DVE involvement.
  Phase B: 64-step recurrence, gates on partitions, batch on free dim.
           Two 32-sentence half-chains ping-pong so the elementwise chain of
           one half hides under the matmul block of the other. Gate chunk
           order [i0 i1 f0 f1 o0 o1 g0 g1]; tanh is computed via the
           sigmoid-only identity tanh(x) = 2*sigmoid(2x)-1 (the 2x is folded
           into the g rows of the table and W_hh on the host), so each half
           needs just two ACTIVATE ops. Running max instead of storing h.
  Phase C: transpose the running max, DMA out [64, 256].

Scan order: token s of chain b reads token_ids[b, s] (fwd) or
token_ids[b, len_b-1-s] (bwd), pad row for s >= len_b, so pads always trail
and both directions share the same program.
"""

import numpy as np
import ml_dtypes

import concourse.bass as bass
import concourse.bacc as bacc
import concourse.mybir as mybir
import concourse.tile as tile
from concourse import bass_utils

V, E, HID, B, T = 50000, 300, 256, 256, 64
NCORES = 8
NSC = 64                    # sentences per core (one direction)
NTOK = NSC * T              # 4096 tokens/core
NTT = NTOK // 128           # 32 gather tiles
G4 = 4 * HID                # 1024 gate logits
VP = V + 1                  # table rows (last = pad row)
BIG = 30.0

F32 = mybir.dt.float32
BF16 = mybir.dt.bfloat16
I32 = mybir.dt.int32
AF = mybir.ActivationFunctionType
OP = mybir.AluOpType

bf = ml_dtypes.bfloat16

_CACHE = {}
LAST_RESULTS = None


def _build_program():
    nc = bacc.Bacc(None, target_bir_lowering=False)

    tab_d = nc.dram_tensor("tab", [VP, G4], BF16, kind="ExternalInput")
    idx_d = nc.dram_tensor("idx", [128, NTT], I32, kind="ExternalInput")
    wstat_d = nc.dram_tensor("wstat", [128, 2048], BF16, kind="ExternalInput")
    out_d = nc.dram_tensor("out", [NSC, HID], F32, kind="ExternalOutput")

    with tile.TileContext(nc) as tc:
        with (
            tc.tile_pool(name="const", bufs=1) as cpool,
            tc.tile_pool(name="stage", bufs=4) as spool,
            tc.tile_pool(name="work", bufs=3) as wpool,
            tc.tile_pool(name="psum", bufs=2, space="PSUM") as ppool,
            tc.tile_pool(name="psumt", bufs=2, space="PSUM") as tpool,
        ):
            wstat_sb = cpool.tile([128, 2048], BF16, tag="wstat")
            idx_sb = cpool.tile([128, NTT], I32, tag="idx")
            zxT = cpool.tile([128, T * 512], BF16, tag="zxT")
            h_pp = [cpool.tile([128, 128], BF16, tag=f"h{i}", name=f"h{i}")
                    for i in range(2)]
            c_st = cpool.tile([128, 128], F32, tag="c_st")
            rmax = cpool.tile([128, 128], BF16, tag="rmax")
            ident = cpool.tile([128, 128], F32, tag="ident")
            ident_bf = cpool.tile([128, 128], BF16, tag="ident_bf")
            outT = cpool.tile([128, 128], F32, tag="outT")

            nc.sync.dma_start(out=wstat_sb[:], in_=wstat_d[:, :])
            nc.sync.dma_start(out=idx_sb[:], in_=idx_d[:, :])

            nc.vector.memset(c_st[:], 0.0)
            nc.vector.memset(h_pp[0][:], 0.0)
            nc.vector.memset(h_pp[1][:], 0.0)
            nc.vector.memset(rmax[:], -1.0)
            from concourse.masks import make_identity
            make_identity(nc, ident[:])
            nc.vector.tensor_copy(out=ident_bf[:], in_=ident[:])

            # ---- Phase A: gather + PE-transpose to gate-major ----
            # zxT col = (s//2)*1024 + ch*128 + (s%2)*64 + b. Gather tile u
            # holds tokens j = u*128 + p, j = s*64 + b. The 8 PE transposes
            # per tile batch through one PSUM bank, then one DMA moves the
            # bank to zxT. Emission is interleaved into the step loop so the
            # in-order PE queue treats transposes as inter-block filler.
            _z = zxT[:]

            def emit_tile(u):
                st = spool.tile([128, G4], BF16, tag="st", name="st")
                nc.gpsimd.indirect_dma_start(
                    out=st[:],
                    out_offset=None,
                    in_=tab_d[:, :],
                    in_offset=bass.IndirectOffsetOnAxis(
                        ap=idx_sb[:, u:u + 1], axis=0),
                )
                tb = tpool.tile([128, 1024], BF16, tag="tb", name="tb")
                for ch in range(8):
                    nc.tensor.transpose(
                        tb[:, ch * 128:(ch + 1) * 128],
                        st[:, ch * 128:(ch + 1) * 128],
                        ident_bf[:])
                if u % 2 == 0:
                    nc.scalar.copy(
                        out=zxT[:, u * 1024:(u + 1) * 1024], in_=tb[:])
                else:
                    nc.vector.tensor_copy(
                        out=zxT[:, u * 1024:(u + 1) * 1024], in_=tb[:])

            LOOKAHEAD = 4
            for u in range(LOOKAHEAD):
                emit_tile(u)

            # ---- Phase B: recurrence ----
            for s in range(T):
                if s % 2 == 0 and s // 2 + LOOKAHEAD < NTT:
                    emit_tile(s // 2 + LOOKAHEAD)
                h_prev = h_pp[(s + 1) % 2]
                h_cur = h_pp[s % 2]
                for half in range(2):
                    co = half * 64
                    zqb = ppool.tile([128, 256], F32, tag=f"zqb{half}")
                    # zxT col = (s//2)*1024 + ch*128 + (s%2)*64 + b
                    zx_ap = bass.AP(
                        tensor=_z.tensor,
                        offset=(_z.offset + (s // 2) * 1024 + (s % 2) * 64
                                + half * 32),
                        ap=[_z.ap[0], [128, 8], [1, 32]])
                    # inject zx into psum via identity matmul, then accumulate
                    nc.tensor.matmul(
                        zqb[:], lhsT=ident_bf[:], rhs=zx_ap,
                        start=True, stop=False, skip_group_check=True)
                    for ch in range(8):
                        for k in range(2):
                            rhs = h_prev[:, co + k * 32:co + k * 32 + 32]
                            nc.tensor.matmul(
                                zqb[:, ch * 32:(ch + 1) * 32],
                                lhsT=wstat_sb[:, (ch * 2 + k) * 128:
                                              (ch * 2 + k + 1) * 128],
                                rhs=rhs,
                                start=False, stop=(k == 1),
                                skip_group_check=True,
                            )
                    sg = wpool.tile([128, 256], F32, tag=f"sg{half}")
                    nc.scalar.activation(sg[:, 0:192], zqb[:, 0:192],
                                         AF.Sigmoid)
                    nc.scalar.activation(sg[:, 192:256], zqb[:, 192:256],
                                         AF.Tanh)
                    tig = wpool.tile([128, 64], F32, tag=f"tig{half}")
                    nc.vector.tensor_mul(tig[:], sg[:, 0:64], sg[:, 192:256])
                    cc = c_st[:, co:co + 64]
                    nc.vector.tensor_mul(cc, cc, sg[:, 64:128])
                    nc.vector.tensor_add(cc, cc, tig[:])
                    th = wpool.tile([128, 64], F32, tag=f"th{half}")
                    nc.scalar.activation(th[:], cc, AF.Tanh)
                    hh = h_cur[:, co:co + 64]
                    nc.vector.tensor_mul(hh, sg[:, 128:192], th[:])
                    nc.vector.tensor_max(rmax[:, co:co + 64],
                                         rmax[:, co:co + 64], hh)

            # ---- Phase C: transpose running max, write out ----
            tp = tpool.tile([128, 128], BF16, tag="tp")
            nc.tensor.transpose(tp[:], rmax[:], ident_bf[:])
            nc.vector.tensor_copy(out=outT[:], in_=tp[:])
            # outT[j = half*64 + k*32 + b, p] -> out[half*32 + b, k*128 + p]
            for half in range(2):
                for k in range(2):
                    out_ap = bass.AP(
                        tensor=out_d[:, :].tensor,
                        offset=(half * 32) * HID + k * 128,
                        ap=[[HID, 32], [1, 128]])
                    nc.sync.dma_start(
                        out=out_ap,
                        in_=outT[half * 64 + k * 32:half * 64 + k * 32 + 32, :])

    nc.finalize()
    return nc


def _host_prep(token_ids, lengths, emb, w_ih_f, w_hh_f, b_f, w_ih_b, w_hh_b,
               b_b):
    # gate chunk order [i0 i1 f0 f1 o0 o1 g0 g1] as torch rows
    ch_rows = [0, 128, 256, 384, 768, 896, 512, 640]
    col_perm = np.concatenate([np.arange(r, r + 128) for r in ch_rows])

    tabs, wstats = {}, {}
    for d in range(2):
        w_ih = w_ih_f if d == 0 else w_ih_b
        whh = w_hh_f if d == 0 else w_hh_b
        bias = b_f if d == 0 else b_b

        zx = emb.astype(np.float32) @ w_ih.T.astype(np.float32) + bias
        tab = np.empty((VP, G4), dtype=bf)
        tab[:V] = zx[:, col_perm].astype(bf)
        padv = np.empty(G4, dtype=np.float32)       # in permuted chunk order
        padv[0:256] = BIG                           # i -> 1
        padv[256:512] = -BIG                        # f -> 0
        padv[512:768] = BIG                         # o -> 1
        padv[768:1024] = -BIG                       # g -> tanh = -1
        tab[V] = padv.astype(bf)
        tabs[d] = tab

        whh2 = whh.astype(np.float32)
        wstat = np.zeros((128, 2048), dtype=bf)
        for ch in range(8):
            for k in range(2):
                blk = whh2[ch_rows[ch]:ch_rows[ch] + 128,
                           k * 128:(k + 1) * 128].T
                col = (ch * 2 + k) * 128
                wstat[:, col:col + 128] = blk.astype(bf)
        wstats[d] = wstat

    in_maps = []
    for c in range(NCORES):
        d = 0 if c < 4 else 1
        blk = c % 4
        tok = token_ids[blk * NSC:(blk + 1) * NSC]      # [64, 64]
        ln = lengths[blk * NSC:(blk + 1) * NSC]         # [64]

        ss = np.arange(T)[None, :]                      # [1, T]
        if d == 0:
            pos = ss                                    # fwd: s
        else:
            pos = ln[:, None] - 1 - ss                  # bwd: len-1-s
        valid = ss < ln[:, None]                        # [64, T]
        rows = np.where(valid, np.take_along_axis(
            tok, np.clip(pos, 0, T - 1), axis=1), V)    # [64, T] table rows
        flat = rows.T.reshape(-1)                       # j = s*64 + b
        idx = flat.reshape(NTT, 128).T.astype(np.int32).copy()

        in_maps.append({
            "tab": tabs[d],
            "idx": idx,
            "wstat": wstats[d],
        })
    return in_maps


def kernel(token_ids, lengths, emb, w_ih_f, w_hh_f, b_f, w_ih_b, w_hh_b, b_b):
    global LAST_RESULTS
    if "nc" not in _CACHE:
        _CACHE["nc"] = _build_program()
    nc = _CACHE["nc"]
    in_maps = _host_prep(token_ids, lengths, emb, w_ih_f, w_hh_f, b_f,
                         w_ih_b, w_hh_b, b_b)
    res = bass_utils.run_bass_kernel_spmd(nc, in_maps, list(range(NCORES)))
    LAST_RESULTS = res
    out = np.zeros((B, 2 * HID), np.float32)
    for c in range(NCORES):
        d = 0 if c < 4 else 1
        blk = c % 4
        out[blk * NSC:(blk + 1) * NSC,
            d * HID:(d + 1) * HID] = res.results[c]["out"]
    return out
